# revision 5
# baseline (speedup 1.0000x reference)
import numpy as np
import ml_dtypes

BF = ml_dtypes.bfloat16
B, N, WT, F, H = 64, 512, 24, 16, 128
NL = N // 8  # nodes per core


def _build(nc, bass, mybir, tile):
    f32 = mybir.dt.float32
    bf16 = mybir.dt.bfloat16
    Alu = mybir.AluOpType
    Act = mybir.ActivationFunctionType

    t_a1t = nc.dram_tensor("a1t", [128, 4, NL], bf16, kind="ExternalInput").ap()
    t_a2t = nc.dram_tensor("a2t", [128, 4, NL], bf16, kind="ExternalInput").ap()
    t_xm = nc.dram_tensor("xm", [B, 128, 4, WT * F], bf16, kind="ExternalInput").ap()
    t_xc0 = nc.dram_tensor("xc0", [128, 3, B, NL], bf16, kind="ExternalInput").ap()
    t_dc = nc.dram_tensor("dc", [128, 8, 3, 128], bf16, kind="ExternalInput").ap()
    t_wih = nc.dram_tensor("wih", [128, NL, 3, 128], bf16, kind="ExternalInput").ap()
    t_whh = nc.dram_tensor("whh", [128, NL, 3, 128], bf16, kind="ExternalInput").ap()
    t_brz = nc.dram_tensor("brz", [128, 2, NL], f32, kind="ExternalInput").ap()
    t_bin = nc.dram_tensor("bin", [128, NL], f32, kind="ExternalInput").ap()
    t_bhn = nc.dram_tensor("bhn", [128, NL], f32, kind="ExternalInput").ap()
    t_wout = nc.dram_tensor("wout", [128, F], bf16, kind="ExternalInput").ap()
    t_bout = nc.dram_tensor("bout", [128, F], f32, kind="ExternalInput").ap()
    t_out = nc.dram_tensor("out", [128, 32, F], f32, kind="ExternalOutput").ap()
    # spills of hop-1 / hop-2 diffusion results, [part, cc, b, n]
    t_x1d = nc.dram_tensor("x1d", [128, 3, B, NL], bf16).ap()
    t_x2d = nc.dram_tensor("x2d", [128, 3, B, NL], bf16).ap()

    with tile.TileContext(nc) as tc:
        with (
            tc.tile_pool(name="const", bufs=1) as cpool,
            tc.tile_pool(name="hpool", bufs=2) as hpool,
            tc.tile_pool(name="stage", bufs=1) as spool,
        ):
            dc = cpool.tile([128, 8, 3, 128], bf16)
            wih = cpool.tile([128, NL, 3, 128], bf16)
            whh = cpool.tile([128, NL, 3, 128], bf16)
            brz = cpool.tile([128, 2, NL], f32)
            bin_ = cpool.tile([128, NL], f32)
            bhn = cpool.tile([128, NL], f32)
            wout = cpool.tile([128, F], bf16)
            bout = cpool.tile([128, F], f32)
            for sb, dr in [(dc, t_dc), (wih, t_wih), (whh, t_whh), (brz, t_brz),
                           (bin_, t_bin), (bhn, t_bhn), (wout, t_wout),
                           (bout, t_bout)]:
                nc.sync.dma_start(sb[:], dr[:])

            # h state, two half tiles (nodes 0:32, 32:64), double buffered
            h_cur = [hpool.tile([128, B, 32], bf16, tag=f"h{i}", name=f"h{i}") for i in range(2)]
            nc.any.memset(h_cur[0][:], 0.0)
            nc.any.memset(h_cur[1][:], 0.0)

            # ---- phase 1: diffusion hops (A, A^2) spilled to DRAM ----
            with (
                tc.tile_pool(name="apool", bufs=1) as apool,
                tc.tile_pool(name="xmp", bufs=3) as xmpool,
                tc.tile_pool(name="p1", bufs=2, space="PSUM") as p1pool,
                tc.tile_pool(name="ev1", bufs=3) as evpool,
            ):
                a1t = apool.tile([128, 4, NL], bf16)
                a2t = apool.tile([128, 4, NL], bf16)
                nc.sync.dma_start(a1t[:], t_a1t[:])
                nc.sync.dma_start(a2t[:], t_a2t[:])
                for bp in range(32):
                    xmb = xmpool.tile([128, 2, 4, WT * F], bf16)
                    nc.sync.dma_start(xmb[:, 0], t_xm[2 * bp])
                    nc.sync.dma_start(xmb[:, 1], t_xm[2 * bp + 1])
                    P1 = p1pool.tile([128, 3, NL, 2], f32, tag="P1")
                    P2 = p1pool.tile([128, 3, NL, 2], f32, tag="P2")
                    for sub in range(2):
                        for cc in range(3):
                            for mc in range(4):
                                lhsT = xmb[:, sub, mc, 128 * cc:128 * cc + 128]
                                nc.tensor.matmul(P1[:, cc, :, sub], lhsT,
                                                 a1t[:, mc, :],
                                                 start=(mc == 0), stop=(mc == 3))
                                nc.tensor.matmul(P2[:, cc, :, sub], lhsT,
                                                 a2t[:, mc, :],
                                                 start=(mc == 0), stop=(mc == 3))
                    e1 = evpool.tile([128, 3, 2, NL], bf16, tag="e1")
                    e2 = evpool.tile([128, 3, 2, NL], bf16, tag="e2")
                    # transpose (n, b) -> (b, n) during psum evacuation
                    nc.vector.tensor_copy(e1[:], P1[:].rearrange("p c n b -> p c b n"))
                    nc.scalar.copy(e2[:], P2[:].rearrange("p c n b -> p c b n"))
                    nc.sync.dma_start(t_x1d[:, :, 2 * bp:2 * bp + 2, :], e1[:])
                    nc.sync.dma_start(t_x2d[:, :, 2 * bp:2 * bp + 2, :], e2[:])

            # ---- phase 2: fused projection + GRU over time ----
            with (
                tc.tile_pool(name="xcw", bufs=1) as xcpool,
                tc.tile_pool(name="spw", bufs=1) as sppool,
                tc.tile_pool(name="gp", bufs=4, space="PSUM") as gpsum,
            ):
                xcw = None
                sp_w = sppool.tile([128, B, NL], bf16)
                sp_flat = sp_w[:].rearrange("p b n -> p (b n)")

                def load_xc(cc):
                    x0 = xcpool.tile([128, B, NL], bf16, tag="xc0")
                    x1 = xcpool.tile([128, B, NL], bf16, tag="xc1")
                    x2 = xcpool.tile([128, B, NL], bf16, tag="xc2")
                    nc.sync.dma_start(x0[:], t_xc0[:, cc])
                    nc.sync.dma_start(x1[:], t_x1d[:, cc])
                    nc.sync.dma_start(x2[:], t_x2d[:, cc])
                    return [x0, x1, x2]

                def proj(w, xcs):
                    # sp[h; b, n] = sum_hop dc_hop^T @ xc_hop ; free chunks of 1024
                    wo = w % 8
                    flats = [xc[:].rearrange("p b n -> p (b n)") for xc in xcs]
                    for ch in range(4):
                        Pp0 = gpsum.tile([128, 2, 8, B], f32, tag="g", name="Pp0")
                        Pp = Pp0[:].rearrange("p t j b -> p (t j b)")
                        sl = slice(1024 * ch, 1024 * ch + 1024)
                        for hop in range(3):
                            rhs = flats[hop][:, sl]
                            for q in range(2):
                                nc.tensor.matmul(
                                    Pp[:, 512 * q:512 * q + 512],
                                    dc[:, wo, hop, :],
                                    rhs[:, 512 * q:512 * q + 512],
                                    start=(hop == 0), stop=(hop == 2))
                        nc.scalar.copy(sp_flat[:, sl], Pp[:])

                # sp for w=0
                xcw = load_xc(0)
                proj(0, xcw)

                for w in range(WT):
                    h_new = [hpool.tile([128, B, 32], bf16, tag=f"h{i}", name=f"hn{i}")
                             for i in range(2)]
                    spv = sp_w
                    rzpre = [spool.tile([128, 2, B, 32], bf16, tag=f"rzp{i}", name=f"rzp{i}")
                             for i in range(2)]
                    t2 = [spool.tile([128, B, 32], bf16, tag=f"t2{i}", name=f"t2{i}")
                          for i in range(2)]
                    t4 = [spool.tile([128, B, 32], bf16, tag=f"t4{i}", name=f"t4{i}")
                          for i in range(2)]
                    nt = [spool.tile([128, B, 32], bf16, tag=f"nt{i}", name=f"nt{i}")
                          for i in range(2)]
                    hm = [spool.tile([128, B, 32], bf16, tag=f"hm{i}", name=f"hm{i}")
                          for i in range(2)]

                    # --- gate matmuls + psum evacuation, groups of 8 nodes ---
                    for g in range(8):
                        half = g // 4
                        n0 = 8 * g
                        RZ = gpsum.tile([128, 2, 8, B], f32, tag="g", name="RZ")
                        for j in range(8):
                            n = n0 + j
                            sp_n = spv[:, :, n]
                            h_n = h_cur[half][:, :, n - 32 * half]
                            for gc in range(2):
                                o = RZ[:, gc, j, :]
                                nc.tensor.matmul(o, wih[:, n, gc, :], sp_n,
                                                 start=True, stop=False)
                                nc.tensor.matmul(o, whh[:, n, gc, :], h_n,
                                                 start=False, stop=True)
                        # rz-pre = psum + brz, to bf16 SBUF [2, b, 8n]
                        dst = rzpre[half][:, :, :, n0 - 32 * half:n0 - 32 * half + 8]
                        src = RZ[:].rearrange("p t j b -> p t b j")
                        bias = brz[:, :, None, n0:n0 + 8].to_broadcast((128, 2, B, 8))
                        nc.vector.tensor_tensor(dst, src, bias, Alu.add)
                    for g in range(8):
                        half = g // 4
                        n0 = 8 * g
                        GN = gpsum.tile([128, 2, 8, B], f32, tag="g", name="GN")
                        GI = GN[:, 0]
                        GH = GN[:, 1]
                        for j in range(8):
                            n = n0 + j
                            sp_n = spv[:, :, n]
                            h_n = h_cur[half][:, :, n - 32 * half]
                            nc.tensor.matmul(GN[:, 0, j, :], wih[:, n, 2, :], sp_n,
                                             start=True, stop=True)
                            nc.tensor.matmul(GN[:, 1, j, :], whh[:, n, 2, :], h_n,
                                             start=True, stop=True)
                        o0 = n0 - 32 * half
                        d4 = t4[half][:, :, o0:o0 + 8]
                        d2 = t2[half][:, :, o0:o0 + 8]
                        bi = bin_[:, None, n0:n0 + 8].to_broadcast((128, B, 8))
                        bh = bhn[:, None, n0:n0 + 8].to_broadcast((128, B, 8))
                        gi_s = GN[:, 0].rearrange("p j b -> p b j")
                        gh_s = GN[:, 1].rearrange("p j b -> p b j")
                        nc.vector.tensor_tensor(d4, gi_s, bi, Alu.add)
                        nc.vector.tensor_tensor(d2, gh_s, bh, Alu.add)

                    # --- per-half activations + gate combine ---
                    for half in range(2):
                        rp = rzpre[half]
                        zc = nt[half]  # reuse nt buffer early for 1-z
                        nc.scalar.activation(zc[:], rp[:, 1], Act.Sigmoid,
                                             scale=-1.0)
                        nc.scalar.activation(rp[:], rp[:], Act.Sigmoid)
                        r = rp[:, 0]
                        z = rp[:, 1]
                        # zh = z * h, off the tanh critical path
                        hmj = hm[half]
                        nc.vector.tensor_tensor(hmj[:], z, h_cur[half][:],
                                                Alu.mult)
                        tt = t2[half]
                        nc.vector.tensor_tensor(tt[:], r, tt[:], Alu.mult)
                        nc.vector.tensor_tensor(tt[:], tt[:], t4[half][:], Alu.add)
                        nc.scalar.activation(tt[:], tt[:], Act.Tanh)
                        nc.vector.tensor_tensor(zc[:], zc[:], tt[:], Alu.mult)
                        nc.vector.tensor_tensor(h_new[half][:], zc[:],
                                                hmj[:], Alu.add)
                    h_cur = h_new

                    # --- prepare sp for next step (overlaps with this step) ---
                    if w + 1 < WT:
                        if (w + 1) % 8 == 0:
                            xcw = load_xc((w + 1) // 8)
                        proj(w + 1, xcw)

            # ---- output projection ----
            with (
                tc.tile_pool(name="po", bufs=1, space="PSUM") as popool,
                tc.tile_pool(name="ou", bufs=1) as oupool,
            ):
                Po = popool.tile([128, 32, F], f32)
                hpk = oupool.tile([128, 2, 16, 128], bf16)
                for half in range(2):
                    # pack [b, (c nh)] -> [c, (b nh)] so lhsT has one free dim
                    nc.vector.tensor_copy(
                        hpk[:, half].rearrange("p c (b nh) -> p c b nh", b=B),
                        h_cur[half][:].rearrange("p b (c nh) -> p c b nh", c=16))
                for c in range(32):
                    nc.tensor.matmul(Po[:, c, :], hpk[:, c // 16, c % 16, :],
                                     wout[:], start=True, stop=True)
                outsb = oupool.tile([128, 32, F], f32)
                nc.vector.tensor_tensor(
                    outsb[:], Po[:], bout[:, None, :].to_broadcast((128, 32, F)),
                    Alu.add)
                nc.sync.dma_start(t_out[:], outsb[:])
    nc.compile()


def kernel(**inputs):
    import concourse.bacc as bacc
    import concourse.bass as bass
    import concourse.mybir as mybir
    import concourse.tile as tile
    from concourse import bass_utils

    x = np.asarray(inputs["x"], np.float32)
    A = np.asarray(inputs["A_fw"], np.float32)
    dcw = np.asarray(inputs["dc_weights"], np.float32)
    W_ih = np.asarray(inputs["W_ih"], np.float32)
    W_hh = np.asarray(inputs["W_hh"], np.float32)
    b_ih = np.asarray(inputs["b_ih"], np.float32)
    b_hh = np.asarray(inputs["b_hh"], np.float32)
    W_out = np.asarray(inputs["W_out"], np.float32)
    b_out = np.asarray(inputs["b_out"], np.float32)

    A2 = A @ A
    dc_all = np.stack([dcw[0:16], dcw[16:32] + dcw[32:48], dcw[48:64] + dcw[64:80]])
    xbf = x.astype(BF)
    xm = np.ascontiguousarray(xbf.reshape(B, 4, 128, WT * F).transpose(0, 2, 1, 3))
    dcm = np.zeros((128, 8, 3, 128), np.float32)
    for wo in range(8):
        dcm[wo * 16:wo * 16 + 16, wo] = dc_all.transpose(1, 0, 2)
    dc_host = dcm.astype(BF)
    wout_h = W_out.astype(BF)
    bout_h = np.tile(b_out[None, :], (128, 1)).astype(np.float32)

    in_maps = []
    for c in range(8):
        ns = slice(c * NL, (c + 1) * NL)
        a1t = np.ascontiguousarray(
            A[ns].T.astype(BF).reshape(4, 128, NL).transpose(1, 0, 2))
        a2t = np.ascontiguousarray(
            A2[ns].T.astype(BF).reshape(4, 128, NL).transpose(1, 0, 2))
        xl = xbf[:, ns]  # [b, n, w, f]
        xc0 = np.ascontiguousarray(
            xl.reshape(B, NL, 3, 8, F).transpose(3, 4, 2, 0, 1)
            .reshape(128, 3, B, NL))
        wih_h = np.ascontiguousarray(
            W_ih[ns].transpose(2, 0, 1).astype(BF).reshape(128, NL, 3, 128))
        whh_h = np.ascontiguousarray(
            W_hh[ns].transpose(2, 0, 1).astype(BF).reshape(128, NL, 3, 128))
        br = (b_ih[ns, 0:128] + b_hh[ns, 0:128]).T
        bz = (b_ih[ns, 128:256] + b_hh[ns, 128:256]).T
        brz_h = np.ascontiguousarray(np.stack([br, bz], axis=1)).astype(np.float32)
        bin_h = np.ascontiguousarray(b_ih[ns, 256:384].T).astype(np.float32)
        bhn_h = np.ascontiguousarray(b_hh[ns, 256:384].T).astype(np.float32)
        in_maps.append({
            "a1t": a1t, "a2t": a2t, "xm": xm, "xc0": xc0, "dc": dc_host,
            "wih": wih_h, "whh": whh_h, "brz": brz_h, "bin": bin_h, "bhn": bhn_h,
            "wout": wout_h, "bout": bout_h,
        })

    nc = bacc.Bacc("TRN2", target_bir_lowering=False, debug=False, num_devices=8)
    _build(nc, bass, mybir, tile)
    res = bass_utils.run_bass_kernel_spmd(nc, in_maps, core_ids=list(range(8)))

    out = np.zeros((B, N, F), np.float32)
    for c in range(8):
        arr = res.results[c]["out"]  # [128 (b*2+nhat), 32 c, F]
        tmp = arr.reshape(B, 2, 32, F).transpose(0, 2, 1, 3).reshape(B, NL, F)
        out[:, c * NL:(c + 1) * NL] = tmp
    return out
